# revision 47
# baseline (speedup 1.0000x reference)
"""Multi-head attention with additive positional attention — TRN2 Bass kernel.

Problem: B=4, S=2048, DM=128, H=8, DK=16.
  scores = (q @ k^T) / sqrt(DK) + pos_q @ pos_k^T   per (b, h)
  out    = softmax(scores) @ v, heads merged, @ Wo^T + bo

Sharding: 8 cores = batch (4) x query-row halves (2). Each core holds one
batch's full keys/values (S=2048) and 1024 query rows, computes all 8 heads,
and produces complete output rows — no cross-core reduction; the host gather
is a pure concatenation.

v3 design (all matmuls bf16, 1 cyc/row on the PE; rel_fro ~1.3e-2 vs the
2e-2 gate, validated in numpy emulation and on HW):
  - host packs every activation input into ONE bf16 DRAM tensor (xall) so
    the input load is a single near-peak-bandwidth DMA; weights/biases are
    packed the same way.
  - kcat/qcat: per head h, a 32-partition block [k_h (16 rows); pos_k_h]
    (resp. [q_h * scale; pos_q_h]) so scoresT = kcat_blk^T @ qcat_blk fuses
    the qk and positional terms into ONE K=32 matmul per tile, row-tiled on
    the PE's 32-row groups. Interleave = 8 nested-partition-AP DMAs.
  - main loop over (head-pair gg, key-chunk kc, query-chunk qc): score tile
    [128 keys, 2 heads x 512 q] (2 PSUM banks, 3-buffered), exp split
    between the scalar engine (exact, cols [0:CA)) and the vector engine
    (Schraudolph fast-exp writing bf16 bit patterns via int16, cols
    [CA:1024)), then attn@v accumulated IN PSUM across all 16 key chunks
    (start/stop flags) — no vector-engine adds, and the score banks recycle
    as soon as exp has read them, so PE/ACT/DVE all stream.
  - v is augmented to 32 columns per head [1 | v_h | 0*15]: attn@v, the
    softmax row-sums, and hard zeros for the padding rows come from one
    col-tiled matmul (M=32). The two avp PSUM accumulators (one per qc)
    each collect FOUR heads (32-row blocks), so one copy per qc evacuates
    a whole half of the heads in the baseline xs layout.
  - normalization: row-sums -> 1/x via exp(-ln(x*2^-32) - 32 ln2) on the
    scalar engine (the Ln spline misbehaves above ~1e18; sums reach ~1e23),
    broadcast to head-blocks by DRAM-bounce DMA, one multiply per half.
  - output projection reads the scattered [head-block] layout through a
    host-permuted Wo, accumulating both halves into one PSUM tile.
"""

import numpy as np

H, DK, DM = 8, 16, 128
B, S = 4, 2048
R = 1024  # query rows per core
NCORES = 8
NKC = S // 128  # 16 key chunks
NQC = 2  # query chunks of 512

# xall column layout
XK0, XP0, XV0, XQ0, XPQ0 = 0, S, 2 * S, 3 * S, 3 * S + R
XCOLS = 3 * S + 2 * R

# exp split: cols [0:CA) on the scalar engine, [CA:1024) Schraudolph on DVE
CA = 560
# bf16 Schraudolph: bits_bf16(exp(x)) ~= int16(x * 128/ln2 + (16256 - c))
SCHR_A = float(128.0 / np.log(2.0))
SCHR_B = 16256.0 - 8.0

DEBUG = False

_CACHE = {}


def _patch_drain():
    """walrus on this stack rejects >1 sync-wait on CTRL instructions; the
    TileContext exit drain can carry several. Absorb them on SP nops first."""
    import concourse.mybir as mybir
    from concourse.tile import TileContext, ScopedClock

    if getattr(TileContext, "_drain_patched", False):
        return
    orig = TileContext._drain_and_barrier

    def patched(self, tick_clock, wait_clock):
        nc = self.nc
        probe = nc.sync.nop(nofuse=True)
        wait_clock.add_sem_waits(
            probe.ins, ScopedClock({None: tick_clock.global_clock})
        )
        w = list(probe.ins.sync_info.on_wait or []) if probe.ins.sync_info else []
        if len(w) > 1:
            probe.ins.sync_info.on_wait = w[:1]
            for i in range(1, len(w)):
                n2 = nc.sync.nop(nofuse=True)
                n2.ins.sync_info = mybir.SyncInfo(on_wait=w[i : i + 1], on_update=[])

        class _NoWaits:
            def __init__(s, real):
                s._real = real

            def add_sem_waits(s, ins, clock):
                pass

            def __getattr__(s, k):
                return getattr(s._real, k)

        orig(self, tick_clock, _NoWaits(wait_clock))

    TileContext._drain_and_barrier = patched
    TileContext._drain_patched = True


def _split_multi_waits(nc, mybir):
    """walrus here accepts at most 1 sync-wait on most instruction structs
    (2 on EventSemaphore). Hoist excess waits onto same-engine NoOps placed
    immediately before the instruction — same blocking semantics."""
    for f in nc.m.functions:
        for blk in f.blocks:
            new_insts = []
            changed = False
            for inst in blk.instructions:
                si = inst.sync_info
                waits = list(si.on_wait) if si and si.on_wait else []
                limit = 2 if type(inst).__name__ == "InstEventSemaphore" else 1
                if len(waits) > limit:
                    changed = True
                    extra = waits[: len(waits) - limit]
                    for wv in extra:
                        n = mybir.InstNoOp(
                            name=f"wsplit_{nc.next_id()}",
                            engine=inst.engine,
                            ins=[],
                            outs=[],
                            sync_info=mybir.SyncInfo(on_wait=[wv], on_update=[]),
                        )
                        nc.register_instruction(n)
                        new_insts.append(n)
                    inst.sync_info.on_wait = waits[len(waits) - limit :]
                new_insts.append(inst)
            if changed:
                blk.instructions = new_insts


def build_bass():
    import concourse.bass as bass
    import concourse.mybir as mybir
    import concourse.tile as tile

    _patch_drain()
    dt = mybir.dt
    f32 = dt.float32
    bf16 = dt.bfloat16
    i16 = dt.int16
    AF = mybir.ActivationFunctionType
    OP = mybir.AluOpType

    nc = bass.Bass("TRN2", num_devices=NCORES, enable_asserts=True)

    xall_d = nc.dram_tensor("xall", [DM, XCOLS], bf16, kind="ExternalInput")
    wall_d = nc.dram_tensor("wall", [DM, 4 * DM], bf16, kind="ExternalInput")
    wallo_d = nc.dram_tensor("wallo", [DM, 2 * DM], bf16, kind="ExternalInput")
    ball_d = nc.dram_tensor("ball", [DM, 4 + DM], f32, kind="ExternalInput")
    # selmat[k, 32k+u] = 1: one matmul broadcasts reciprocal row k of r8
    # into the 32-row block of head k (engines can't cross partitions)
    selm_d = nc.dram_tensor("selm", [4, DM], bf16, kind="ExternalInput")
    outT_d = nc.dram_tensor("outT", [DM, R], f32, kind="ExternalOutput")
    if DEBUG:
        dbg_xs0_d = nc.dram_tensor("dbg_xs0", [DM, R], f32, kind="ExternalOutput")
        dbg_sums_d = nc.dram_tensor("dbg_sums", [8, R], f32, kind="ExternalOutput")
        dbg_r8_d = nc.dram_tensor("dbg_r8", [8, R], f32, kind="ExternalOutput")

    with tile.TileContext(nc) as tc:
        with (
            tc.tile_pool(name="singles", bufs=1) as singles,
            tc.tile_pool(name="exps", bufs=6) as exps,
        ):
            # ---------------- input loads (packed) ----------------
            s_wall = singles.tile([DM, 4 * DM], bf16, tag="wall_s", name="wall_s")
            s_wallo = singles.tile([DM, 2 * DM], bf16, tag="wallo_s", name="wallo_s")
            s_ball = singles.tile([DM, 4 + DM], f32, tag="ball_s", name="ball_s")
            s_selm = singles.tile([4, DM], bf16, tag="selm_s", name="selm_s")
            # one SBUF tile per logical input (from the packed DRAM tensor)
            # so each projection waits only on its own DMA. DMA doorbells are
            # per-initiating-engine queues (SP + ACT are the HW-DGE engines):
            # alternate so transfers run on two queues instead of one, and
            # order by who's needed first (weights + k/pos for the first
            # projections, selm/wallo/ball only at the tail).
            s_xk = singles.tile([DM, S], bf16, tag="xk_s", name="xk_s")
            s_xpos = singles.tile([DM, S], bf16, tag="xpos_s", name="xpos_s")
            s_xv = singles.tile([DM, S], bf16, tag="xv_s", name="xv_s")
            s_xqq = singles.tile([DM, 2 * R], bf16, tag="xqq_s", name="xqq_s")
            nc.sync.dma_start(out=s_wall[:, :], in_=wall_d[:, :])
            nc.sync.dma_start(out=s_xk[:, :], in_=xall_d[:, XK0 : XK0 + S])
            nc.sync.dma_start(out=s_xpos[:, :], in_=xall_d[:, XP0 : XP0 + S])
            nc.sync.dma_start(out=s_xqq[:, :], in_=xall_d[:, XQ0 : XQ0 + 2 * R])
            nc.sync.dma_start(out=s_xv[:, :], in_=xall_d[:, XV0 : XV0 + S])
            nc.sync.dma_start(out=s_ball[:, :], in_=ball_d[:, :])
            nc.sync.dma_start(out=s_wallo[:, :], in_=wallo_d[:, :])
            nc.sync.dma_start(out=s_selm[:, :], in_=selm_d[:, :])

            s_xkT = s_xk[:, :]
            s_posT = s_xpos[:, :]
            s_xvT = s_xv[:, :]
            s_xqT = s_xqq[:, 0:R]
            s_posqT = s_xqq[:, R : 2 * R]
            s_w0Ts = s_wall[:, 0:DM]
            s_w0T = s_wall[:, DM : 2 * DM]
            s_w1T = s_wall[:, 2 * DM : 3 * DM]
            s_w2T = s_wall[:, 3 * DM : 4 * DM]
            s_b0s = s_ball[:, 0:1]
            s_b0c = s_ball[:, 1:2]
            s_b1c = s_ball[:, 2:3]
            s_boc = s_ball[:, 3:4]
            s_b2r = s_ball[:, 4 : 4 + DM]

            # one [64, *] tile per head-pair: the first score matmul then
            # waits on just its own pair's 8 interleave DMAs, not all 32
            kcat = [
                singles.tile([64, S], bf16, tag=f"kcat{gg}", name=f"kcat{gg}")
                for gg in range(4)
            ]
            qcat = [
                singles.tile([64, R], bf16, tag=f"qcat{gg}", name=f"qcat{gg}")
                for gg in range(4)
            ]
            # per head: [1 | v_h (16) | 0*15] -> av matmul M=32 writes the
            # softmax row-sum and hard zeros alongside attn@v. One tile per
            # key chunk so the main loop's av only waits on its own chunk.
            v_aug = [
                singles.tile([DM, 32 * H], bf16, tag=f"vaug{t}", name=f"vaug{t}")
                for t in range(NKC)
            ]
            for t in range(NKC):
                nc.gpsimd.memset(v_aug[t][:, :], 0.0)
                nc.gpsimd.memset(
                    v_aug[t].rearrange("p (h u) -> p h u", u=32)[:, :, 0], 1.0
                )

            # ---------------- projections ----------------
            # full feature-major projections into bf16 SBUF scratch, then DMA
            # partition-interleave into the per-head-block kcat/qcat layout
            kT_sb = singles.tile([DM, S], bf16, tag="kT_sb", name="kT_sb")
            pkT_sb = singles.tile([DM, S], bf16, tag="pkT_sb", name="pkT_sb")
            qT_sb = singles.tile([DM, R], bf16, tag="qT_sb", name="qT_sb")
            pqT_sb = singles.tile([DM, R], bf16, tag="pqT_sb", name="pqT_sb")

            with tc.tile_pool(name="proj_psum", bufs=4, space="PSUM") as proj_psum:
                def proj(lhsT, rhs_src, ncols, bias, dst_sb, phase):
                    for ci, c0 in enumerate(range(0, ncols, 512)):
                        pk = proj_psum.tile([128, 512], f32, tag="proj", name="pk")
                        nc.tensor.matmul(
                            out=pk[:, :],
                            lhsT=lhsT,
                            rhs=rhs_src[:, c0 : c0 + 512],
                            start=True,
                            stop=True,
                        )
                        # evac + bias + round to bf16 in one op; alternate
                        # chunks between the scalar and vector engines so
                        # each staging tensor finishes in half the time
                        if (ci + phase) % 2 == 0:
                            nc.scalar.activation(
                                out=dst_sb[:, c0 : c0 + 512],
                                in_=pk[:, :],
                                func=AF.Identity,
                                bias=bias,
                            )
                        else:
                            nc.vector.tensor_scalar_add(
                                out=dst_sb[:, c0 : c0 + 512],
                                in0=pk[:, :],
                                scalar1=bias,
                            )

                proj(s_w1T, s_xkT, S, s_b1c, kT_sb, 0)
                proj(s_w1T, s_posT, S, s_b1c, pkT_sb, 1)
                proj(s_w0Ts, s_xqT, R, s_b0s, qT_sb, 0)
                proj(s_w0T, s_posqT, R, s_b0c, pqT_sb, 1)

                # partition-interleave via SBUF->SBUF DMA (multi-level
                # partition APs do NOT work in DMA descriptors — only the
                # outer dim transfers — so this stays one DMA per 16-row
                # block): kcat[gg][32j+16*half+d] = src[16*(2gg+j)+d],
                # emitted in loop (gg) order so early pairs land first
                for gg in range(4):
                    for j in (0, 1):
                        h = 2 * gg + j
                        for half, ksrc, qsrc in (
                            (0, kT_sb, qT_sb),
                            (1, pkT_sb, pqT_sb),
                        ):
                            r0 = 32 * j + 16 * half
                            nc.sync.dma_start(
                                out=kcat[gg][r0 : r0 + 16, :],
                                in_=ksrc[16 * h : 16 * h + 16, :],
                            )
                            nc.sync.dma_start(
                                out=qcat[gg][r0 : r0 + 16, :],
                                in_=qsrc[16 * h : 16 * h + 16, :],
                            )

                # v projection, seq-major (lhsT = data chunk so the output
                # partition dim is sequence), + bias, into the 32-strided v_aug
                for t in range(NKC):
                    pv = proj_psum.tile([128, 512], f32, tag="proj", name="pv")
                    nc.tensor.matmul(
                        out=pv[:, 0:DM],
                        lhsT=s_xvT[:, t * 128 : (t + 1) * 128],
                        rhs=s_w2T,
                        start=True,
                        stop=True,
                    )
                    nc.vector.tensor_tensor(
                        out=v_aug[t].rearrange("p (h u) -> p h u", u=32)[
                            :, :, 1:17
                        ],
                        in0=pv[:, 0:DM].rearrange("p (h u) -> p h u", u=16),
                        in1=s_b2r.rearrange("p (h u) -> p h u", u=16),
                        op=OP.add,
                    )

            # ---------------- attention main loop ----------------
            ITERS = [
                (gg, kc, qc)
                for gg in range(4)
                for kc in range(NKC)
                for qc in range(NQC)
            ]
            T = len(ITERS)
            sct = {}
            xs = [
                singles.tile([DM, R], f32, tag=f"xs{g4}", name=f"xs{g4}")
                for g4 in range(2)
            ]

            # ---- normalization tiles (chains emitted per half, in-loop) ----
            # separate per-half tiles: ACT/DMA partition access must start
            # at partition 0, so [8,R] slices at partition 4 are not allowed
            sums8 = [
                singles.tile([4, R], f32, tag=f"sums{g4}", name=f"sums{g4}")
                for g4 in range(2)
            ]
            r8 = [
                singles.tile([4, R], bf16, tag=f"r8{g4}", name=f"r8{g4}")
                for g4 in range(2)
            ]
            ln8 = [
                singles.tile([4, R], f32, tag=f"ln8{g4}", name=f"ln8{g4}")
                for g4 in range(2)
            ]
            xsb = [
                singles.tile([DM, R], bf16, tag=f"xsb{g4}", name=f"xsb{g4}")
                for g4 in range(2)
            ]
            nbias = singles.tile([4, 1], f32, tag="nbias", name="nbias")
            nc.gpsimd.memset(nbias[:, :], float(-32.0 * np.log(2.0)))

            def tail_chain(g4):
                """sums -> 1/x for heads 4g4..4g4+3, emitted as soon as the
                half's accumulators are evacuated so g4=0's chain overlaps
                the second half of the main loop.

                1/x = exp(-ln(x*2^-32) - 32*ln2) on the scalar engine: the
                Ln spline misbehaves above ~1e18 and sums reach ~1e23 (scores
                up to ~50, no max-subtraction); vector reciprocal is slower.
                """
                # head h=4*g4+m row-sum lives at xs[g4] row 32m
                nc.sync.dma_start(out=sums8[g4][:, :], in_=xs[g4][0:97:32, :])
                nc.scalar.activation(
                    out=ln8[g4][:, :],
                    in_=sums8[g4][:, :],
                    func=AF.Ln,
                    scale=2.0**-32,
                )
                nc.scalar.activation(
                    out=r8[g4][:, :],
                    in_=ln8[g4][:, :],
                    func=AF.Exp,
                    scale=-1.0,
                    bias=nbias[:, :],
                )

            with (
                tc.tile_pool(name="sc_psum", bufs=3, space="PSUM") as sc_psum,
                tc.tile_pool(name="av_psum", bufs=2, space="PSUM") as av_psum,
            ):
                avp = {}

                def emit_sc(t):
                    gg, kc, qc = ITERS[t]
                    st = sc_psum.tile([128, 2 * 512], f32, tag="sc", name="sc")
                    sct[t] = st
                    for j in (0, 1):
                        r0 = 32 * j
                        nc.tensor.matmul(
                            out=st[:, 512 * j : 512 * (j + 1)],
                            lhsT=kcat[gg][r0 : r0 + 32, kc * 128 : (kc + 1) * 128],
                            rhs=qcat[gg][r0 : r0 + 32, qc * 512 : (qc + 1) * 512],
                            start=True,
                            stop=True,
                            tile_position=(r0, 0),
                        )

                emit_sc(0)
                emit_sc(1)
                for t in range(T):
                    gg, kc, qc = ITERS[t]
                    g4, ee = divmod(gg, 2)
                    st = sct.pop(t)
                    e = exps.tile([128, 2 * 512], bf16, tag="e", name="e")
                    # exact exp on the scalar engine for cols [0:CA)
                    nc.scalar.activation(
                        out=e[:, 0:CA], in_=st[:, 0:CA], func=AF.Exp
                    )
                    # Schraudolph fast-exp on DVE for cols [CA:1024): the
                    # int16 result IS the bf16 bit pattern of exp(x)
                    nc.vector.tensor_scalar(
                        out=e.bitcast(i16)[:, CA : 2 * 512],
                        in0=st[:, CA : 2 * 512],
                        scalar1=SCHR_A,
                        scalar2=SCHR_B,
                        op0=OP.mult,
                        op1=OP.add,
                    )
                    if t + 2 < T:
                        emit_sc(t + 2)
                    # attn @ [1|v|0..]: col-tiled M=32 matmuls accumulating
                    # across all 16 key chunks directly in PSUM; each avp
                    # tile collects 4 heads (ee selects the 64-row half)
                    if gg % 2 == 0 and kc == 0:
                        avp[qc] = av_psum.tile(
                            [128, 512], f32, tag="avp", name="avp"
                        )
                    for j in (0, 1):
                        h = 2 * gg + j
                        m0 = 64 * ee + 32 * j
                        nc.tensor.matmul(
                            out=avp[qc][m0 : m0 + 32, :],
                            lhsT=v_aug[kc][:, 32 * h : 32 * h + 32],
                            rhs=e[:, 512 * j : 512 * (j + 1)],
                            start=(kc == 0),
                            stop=(kc == NKC - 1),
                            tile_position=(0, m0),
                            skip_group_check=True,
                        )
                    if gg % 2 == 1 and kc == NKC - 1:
                        # evacuate the finished accumulator (4 heads x 32aug)
                        av = avp.pop(qc)
                        nc.vector.tensor_scalar_add(
                            out=xs[g4][:, qc * 512 : (qc + 1) * 512],
                            in0=av[:, :],
                            scalar1=0.0,
                        )
                        if qc == NQC - 1:
                            tail_chain(g4)

            # ---------------- normalize + output projection ----------------
            if DEBUG:
                nc.sync.dma_start(out=dbg_xs0_d[:, :], in_=xs[0][:, :])
                for g4 in range(2):
                    s4 = slice(4 * g4, 4 * g4 + 4)
                    nc.sync.dma_start(out=dbg_sums_d[s4, :], in_=sums8[g4][:, :])

            with tc.tile_pool(name="out_psum", bufs=1, space="PSUM") as out_psum:
                # broadcast reciprocal rows into 32-row head blocks via the
                # selmat indicator matmul (PSUM banks are free post-loop),
                # then one multiply per half
                for g4 in range(2):
                    rbp = out_psum.tile([DM, R], f32, tag=f"rbp{g4}", name=f"rbp{g4}")
                    for qc in range(NQC):
                        sl = slice(qc * 512, (qc + 1) * 512)
                        nc.tensor.matmul(
                            out=rbp[:, sl],
                            lhsT=s_selm[:, :],
                            rhs=r8[g4][:, sl],
                            start=True,
                            stop=True,
                        )
                    nc.vector.tensor_tensor(
                        out=xsb[g4][:, :],
                        in0=xs[g4][:, :],
                        in1=rbp[:, :],
                        op=OP.mult,
                    )
                po = out_psum.tile([DM, R], f32, tag="po", name="po")
                for qc in range(NQC):
                    sl = slice(qc * 512, (qc + 1) * 512)
                    for g4 in range(2):
                        nc.tensor.matmul(
                            out=po[:, sl],
                            lhsT=s_wallo[:, g4 * DM : (g4 + 1) * DM],
                            rhs=xsb[g4][:, sl],
                            start=(g4 == 0),
                            stop=(g4 == 1),
                        )
                # bias on the (idle) scalar engine, split per qc half so the
                # first half's output DMA overlaps the second half's bias
                ob = singles.tile([DM, R], f32, tag="ob", name="ob")
                for qc in range(NQC):
                    sl = slice(qc * 512, (qc + 1) * 512)
                    nc.scalar.activation(
                        out=ob[:, sl],
                        in_=po[:, sl],
                        func=AF.Identity,
                        bias=s_boc,
                    )
                    nc.sync.dma_start(out=outT_d[:, sl], in_=ob[:, sl])

    _split_multi_waits(nc, mybir)
    return nc


def shard_inputs(query, key, value, pos_embed, W0, b0, W1, b1, W2, b2, Wo, bo):
    """Build the 8 per-core input maps (host-side layout preprocessing)."""
    import ml_dtypes

    f = np.float32
    bf = ml_dtypes.bfloat16
    asc = np.ascontiguousarray
    scale = 1.0 / np.sqrt(np.float32(DK))

    # wallo[:, g4*128:...][32m+u, f] = Wo^T[16*(4g4+m)+u-1, f] for u in
    # 1..16 (matching the [sum | v | pad] row layout of xsb), else 0
    WoT = Wo.T.astype(f)
    wallo = np.zeros((DM, 2 * DM), f)
    for g4 in range(2):
        for m in range(4):
            h = 4 * g4 + m
            wallo[32 * m + 1 : 32 * m + 17, g4 * DM : (g4 + 1) * DM] = WoT[
                16 * h : 16 * h + 16, :
            ]

    wall = np.concatenate(
        [(W0.T * scale), W0.T, W1.T, W2.T], axis=1
    ).astype(bf)
    ball = np.concatenate(
        [
            (b0 * scale).reshape(DM, 1),
            b0.reshape(DM, 1),
            b1.reshape(DM, 1),
            bo.reshape(DM, 1),
            np.tile(b2.reshape(1, DM), (DM, 1)),
        ],
        axis=1,
    ).astype(f)

    selm = np.zeros((4, DM), f)
    for k in range(4):
        selm[k, 32 * k : 32 * k + 32] = 1.0

    shared = {
        "wall": asc(wall),
        "wallo": asc(wallo.astype(bf)),
        "ball": asc(ball),
        "selm": asc(selm.astype(bf)),
    }
    in_maps = []
    for c in range(NCORES):
        b_i, half = divmod(c, 2)
        r0 = half * R
        xall = np.concatenate(
            [
                key[b_i].T,
                pos_embed[b_i].T,
                value[b_i].T,
                query[b_i, r0 : r0 + R, :].T,
                pos_embed[b_i, r0 : r0 + R, :].T,
            ],
            axis=1,
        ).astype(bf)
        in_maps.append(dict(shared, xall=asc(xall)))
    return in_maps


def gather_outputs(results):
    out = np.empty((B, S, DM), np.float32)
    for c in range(NCORES):
        b_i, half = divmod(c, 2)
        r0 = half * R
        out[b_i, r0 : r0 + R, :] = results[c]["outT"].T
    return out


_TRACE = False  # set by test.py to capture an NTFF profile


def kernel(query, key, value, pos_embed, W0, b0, W1, b1, W2, b2, Wo, bo):
    from concourse.bass_utils import run_bass_kernel_spmd

    if "nc" not in _CACHE:
        _CACHE["nc"] = build_bass()
    in_maps = shard_inputs(
        query, key, value, pos_embed, W0, b0, W1, b1, W2, b2, Wo, bo
    )
    res = run_bass_kernel_spmd(
        _CACHE["nc"],
        in_maps,
        core_ids=list(range(NCORES)),
        trace=_TRACE,
    )
    _CACHE["last_result"] = res
    return gather_outputs(res.results)
